# revision 1
# baseline (speedup 1.0000x reference)
"""Self-contained TRN2 Bass kernel for 16-head MHA (B=2, T=2048, D=1024),
head-parallel across 8 NeuronCores (2 heads per core).

kernel(**inputs) takes the FULL fp32 inputs of reference.setup_inputs() and
returns the FULL [2, 2048, 1024] fp32 output.  Host-side prep: q/k/v are
transposed to [1024, 4096] bf16 (shared by all cores); each core gets its
128-column slice of Wq/Wk/Wv (and 128-row slice of Wo) in bf16.  Each core
computes its two heads end-to-end (QKV projections, softmax attention with
row-group-packed score matmuls, ones-augmented PV for free softmax sums,
output projection) and DMAs a rank-128 partial of the output back; the host
sums the 8 partials and adds the output bias.
"""

import numpy as np

import concourse.bass as bass
import concourse.mybir as mybir
import concourse.tile as tile
from concourse import bacc

FP32 = mybir.dt.float32
FP16 = mybir.dt.float16
BF16 = mybir.dt.bfloat16

D = 1024          # model dim
N = 4096          # B*T tokens
B = 2
T = 2048
PH = 128          # per-core projection dims (2 heads x 64)
DH = 64           # head dim
KC = 8            # contraction chunks (1024 / 128)
NTC = N // 128    # 32 token chunks of 128
SCALE = 0.125     # 1/sqrt(64)

ACT_EXP = mybir.ActivationFunctionType.Exp


def build(nc=None):
    if nc is None:
        nc = bacc.Bacc(
            "TRN2",
            target_bir_lowering=False,
            debug=False,
            enable_asserts=False,
            num_devices=8,
        )

    qT = nc.dram_tensor("qT", [D, N], BF16, kind="ExternalInput")
    kT = nc.dram_tensor("kT", [D, N], BF16, kind="ExternalInput")
    vT = nc.dram_tensor("vT", [D, N], BF16, kind="ExternalInput")
    # host pre-arranges W* into the [128, KC, 128] SBUF layout so the
    # weight DMA is one contiguous 2 KB-per-partition copy
    wq = nc.dram_tensor("wq", [128, KC * PH], BF16, kind="ExternalInput")
    wk = nc.dram_tensor("wk", [128, KC * PH], BF16, kind="ExternalInput")
    wv = nc.dram_tensor("wv", [128, KC * PH], BF16, kind="ExternalInput")
    wo = nc.dram_tensor("wo", [PH, D], BF16, kind="ExternalInput")
    bq = nc.dram_tensor("bq", [PH, 1], FP32, kind="ExternalInput")
    bk = nc.dram_tensor("bk", [PH, 1], FP32, kind="ExternalInput")
    bv = nc.dram_tensor("bv", [PH, 1], FP32, kind="ExternalInput")
    out = nc.dram_tensor("out", [N, D], FP16, kind="ExternalOutput")

    with tile.TileContext(nc) as tc:
        _emit(nc, tc, qT, kT, vT, wq, wk, wv, wo, bq, bk, bv, out)

    nc.compile()
    return nc


class _Ctx:
    pass


def _emit(nc, tc, qT, kT, vT, wq, wk, wv, wo, bq, bk, bv, out):
    from contextlib import ExitStack

    from collections import deque

    E = _Ctx()
    E.nc = nc
    E.fillq = deque()

    ctxmgr = ExitStack()
    with ctxmgr:
        const_pool = ctxmgr.enter_context(tc.tile_pool(name="const", bufs=1))
        E.xt_pool = ctxmgr.enter_context(tc.tile_pool(name="xt", bufs=18))
        big_pool = ctxmgr.enter_context(tc.tile_pool(name="big", bufs=1))
        E.pt_pool = ctxmgr.enter_context(tc.tile_pool(name="pt", bufs=17))
        E.bc_pool = ctxmgr.enter_context(tc.tile_pool(name="bc", bufs=4))
        E.ostg_pool = ctxmgr.enter_context(tc.tile_pool(name="ostg", bufs=6))
        # PSUM: shared proj/outproj pool 2 banks + st 4 + ctx 2 = 8
        E.po_ps = ctxmgr.enter_context(
            tc.tile_pool(name="po_ps", bufs=2, space="PSUM"))
        E.st_ps = ctxmgr.enter_context(
            tc.tile_pool(name="st_ps", bufs=2, space="PSUM"))
        E.ctx_ps = ctxmgr.enter_context(
            tc.tile_pool(name="ctx_ps", bufs=2, space="PSUM"))

        # --- weights / consts to SBUF ---
        wq_sb = const_pool.tile([128, KC, PH], BF16, tag="wq")
        wk_sb = const_pool.tile([128, KC, PH], BF16, tag="wk")
        wv_sb = const_pool.tile([128, KC, PH], BF16, tag="wv")
        E.wo_sb = const_pool.tile([128, D], BF16, tag="wo")
        bq_sb = const_pool.tile([128, 1], FP32, tag="bq")
        bk_sb = const_pool.tile([128, 1], FP32, tag="bk")
        E.bv_sb = const_pool.tile([128, 1], FP32, tag="bv")
        nc.sync.dma_start(wk_sb[:], wk.ap().rearrange("p (c m) -> p c m", c=KC))
        nc.sync.dma_start(bk_sb[:], bk.ap())

        # persistent activations
        E.qT_sb = big_pool.tile([128, N], BF16, tag="qTsb")
        E.kT_sb = big_pool.tile([128, N], BF16, tag="kTsb")
        # v_aug pair layout: [tok part, 32 tok chunks, 130]; per head h the
        # PV stationary operand is vp[:, chunk, 65h : 65h+65] = [v_h | ones]
        E.vp = big_pool.tile([128, NTC, 130], BF16, tag="vp")
        E.vT_sb = big_pool.tile([128, N], BF16, tag="vTsb")
        E.ctxT = big_pool.tile([128, N], BF16, tag="ctxT")

        nc.gpsimd.memset(E.vp[:, :, 64], 1.0)
        nc.gpsimd.memset(E.vp[:, :, 129], 1.0)

        E.identity = const_pool.tile([128, 128], BF16, tag="ident")
        from concourse.masks import make_identity
        make_identity(nc, E.identity[:])

        def dma_in(nm, xdram, b):
            lst = []
            for kc in range(KC):
                xt = E.xt_pool.tile(
                    [128, T], BF16, tag="xt", name=f"xt_{nm}{b}{kc}")
                nc.sync.dma_start(
                    xt[:], xdram.ap()[kc * 128:(kc + 1) * 128, b * T:(b + 1) * T])
                lst.append(xt)
            return lst

        def proj4(xts, wsb, bias_sb, dstT, b, ts=range(4), drain_act=False):
            for t in ts:
                _proj_chunk(E, xts, wsb, bias_sb, dstT, b * T, t, drain_act)

        def vproj(xts, b):
            proj4(xts, wv_sb, None, E.vT_sb, b)
            for tloc in range(16):
                tcid = b * 16 + tloc
                tr = E.po_ps.tile(
                    [128, 128], BF16, tag="po", name=f"tr{tcid}")
                nc.tensor.transpose(
                    tr[:], E.vT_sb[:, tcid * 128:(tcid + 1) * 128], E.identity[:])
                nc.vector.tensor_copy(E.vp[:, tcid, 0:64], tr[:, 0:64])
                nc.vector.tensor_copy(E.vp[:, tcid, 65:129], tr[:, 64:128])

        def group(b, tqc, mid_cb=None):
            return _attention_group(E, b, tqc, mid_cb)

        def pchunk(xts, wsb, bias_sb, dstT, b, t):
            return lambda: _proj_chunk(E, xts, wsb, bias_sb, dstT, b * T, t)

        def unorm(pend, half2):
            # half2==0 also runs the (cheap, DVE-side) normalization
            tq0, sums_h = pend

            def run():
                if half2 == 0:
                    _norm(E, tq0, sums_h)
                _outproj_half(E, tq0, half2, out)
            return run

        # batch-0 inputs + K/Q projections up front; V is emitted after the
        # first attention group so the PE stream does not stall on vT DMA.
        # batch-1 projections and the deferred norm+output-projection units
        # ride the filler queue, paced into the ACT-bound attention slack
        # inside later groups (the PE queue is in-order, so fillers must be
        # interleaved INSIDE groups to cover the per-tk exp-wait bubbles).
        xk0 = dma_in("k", kT, 0)
        nc.sync.dma_start(wq_sb[:], wq.ap().rearrange("p (c m) -> p c m", c=KC))
        nc.sync.dma_start(bq_sb[:], bq.ap())
        xq0 = dma_in("q", qT, 0)
        nc.sync.dma_start(wv_sb[:], wv.ap().rearrange("p (c m) -> p c m", c=KC))
        nc.sync.dma_start(E.wo_sb[:], wo.ap())
        nc.sync.dma_start(E.bv_sb[:], bv.ap())
        xv0 = dma_in("v", vT, 0)
        proj4(xk0, wk_sb, bk_sb, E.kT_sb, 0, drain_act=True)
        proj4(xq0, wq_sb, bq_sb, E.qT_sb, 0, drain_act=True)
        xk1 = dma_in("k", kT, 1)
        g00 = group(0, 0, mid_cb=lambda: vproj(xv0, 0))
        xq1 = dma_in("q", qT, 1)
        E.fillq.extend([
            pchunk(xk1, wk_sb, bk_sb, E.kT_sb, 1, 0),
            pchunk(xk1, wk_sb, bk_sb, E.kT_sb, 1, 1),
            unorm(g00, 0),
        ])
        g01 = group(0, 1)
        xv1 = dma_in("v", vT, 1)
        E.fillq.extend([
            unorm(g00, 1),
            pchunk(xk1, wk_sb, bk_sb, E.kT_sb, 1, 2),
            pchunk(xk1, wk_sb, bk_sb, E.kT_sb, 1, 3),
        ])
        g02 = group(0, 2)
        E.fillq.extend([
            pchunk(xq1, wq_sb, bq_sb, E.qT_sb, 1, 0),
            pchunk(xq1, wq_sb, bq_sb, E.qT_sb, 1, 1),
            pchunk(xq1, wq_sb, bq_sb, E.qT_sb, 1, 2),
        ])
        g03 = group(0, 3)
        # q1's last chunk must land before group(1,0) reads qT batch 1
        _proj_chunk(E, xq1, wq_sb, bq_sb, E.qT_sb, T, 3)
        unorm(g01, 0)()
        E.fillq.extend([
            unorm(g01, 1),
            unorm(g02, 0),
            unorm(g02, 1),
        ])
        g10 = group(1, 0, mid_cb=lambda: vproj(xv1, 1))
        E.fillq.extend([
            unorm(g03, 0),
            unorm(g03, 1),
            unorm(g10, 0),
        ])
        g11 = group(1, 1)
        E.fillq.extend([
            unorm(g10, 1),
            unorm(g11, 0),
            unorm(g11, 1),
        ])
        g12 = group(1, 2)
        E.fillq.extend([
            unorm(g12, 0),
            unorm(g12, 1),
        ])
        g13 = group(1, 3)
        while E.fillq:
            E.fillq.popleft()()
        unorm(g13, 0)()
        unorm(g13, 1)()


def _proj_chunk(E, xts, wsb, bias_sb, dstT, btok, t, drain_act=False):
    """One 512-token projection chunk: accumulate 8 kc matmuls, drain."""
    nc = E.nc
    ps = E.po_ps.tile([128, 512], FP32, tag="po", name="ps")
    for kc in range(KC):
        nc.tensor.matmul(
            ps[:],
            wsb[:, kc, :],
            xts[kc][:, t * 512:(t + 1) * 512],
            start=(kc == 0),
            stop=(kc == KC - 1),
        )
    dst = dstT[:, btok + t * 512: btok + (t + 1) * 512]
    if drain_act:
        # ScalarE drain (idle during the head phase); Identity has a free
        # per-partition bias add
        if bias_sb is not None:
            nc.scalar.activation(
                dst, ps[:], mybir.ActivationFunctionType.Identity, bias=bias_sb[:])
        else:
            nc.scalar.activation(dst, ps[:], mybir.ActivationFunctionType.Identity)
    elif bias_sb is not None:
        nc.vector.tensor_scalar_add(dst, ps[:], bias_sb[:])
    else:
        nc.vector.tensor_copy(dst, ps[:])


def _attention_group(E, b, tqc, mid_cb=None):
    """S^T/exp/PV + sums & ctx drains for one 512-token group (both heads).

    The two heads' S^T matmuls are row-group packed: head h's K=64
    contraction occupies array rows 64h..64h+63, so the pair runs
    concurrently on the PE.  st is one flat [128, 1024] PSUM tile (2 banks,
    head h in columns 512h..512h+512) so the exp is a single contiguous
    2-D ACTIVATE over 1024 elements per partition.

    With mid_cb set, all 16 ST/exp pairs are emitted first, then mid_cb()
    (used for the V projection: ScalarE stays busy on the exps while the
    PE waits for vT's DMA), then the PV accumulation.
    """
    nc = E.nc
    btok = b * T
    tq0 = btok + tqc * 512

    sums_h = [
        E.bc_pool.tile([1, 512], FP32, tag=f"sums{h}", name=f"sums{h}")
        for h in range(2)
    ]
    ctx2 = [
        E.ctx_ps.tile([65, 512], FP32, tag="ctx", name=f"ctx{h}")
        for h in range(2)
    ]

    def st_exp(tk):
        st = E.st_ps.tile([128, 1024], FP32, tag="st", name="st")
        for h in range(2):
            nc.tensor.matmul(
                st[:, h * 512:(h + 1) * 512],
                E.kT_sb[h * 64:(h + 1) * 64,
                        btok + tk * 128: btok + (tk + 1) * 128],
                E.qT_sb[h * 64:(h + 1) * 64, tq0:tq0 + 512],
                start=True,
                stop=True,
            )
        pt = E.pt_pool.tile([128, 1024], BF16, tag="pt", name="pt")
        nc.scalar.activation(pt[:], st[:], ACT_EXP, scale=SCALE)
        return pt

    def pv(tk, pt):
        for h in range(2):
            nc.tensor.matmul(
                ctx2[h][:],
                E.vp[:, b * 16 + tk, h * 65:(h + 1) * 65],
                pt[:, h * 512:(h + 1) * 512],
                start=(tk == 0),
                stop=(tk == 15),
            )

    if mid_cb is None:
        # software pipeline: PV trails the S/exp pair by two tk steps (so
        # each exp has ~2 us before its PV consumes it), and filler units
        # (staggered projections, deferred norm+outproj) are paced into the
        # exp-bound slack at fixed tk slots.
        pipe = []
        for tk in range(16):
            pipe.append(st_exp(tk))
            if len(pipe) > 2:
                pv(tk - 2, pipe.pop(0))
            if tk in (3, 8, 13) and E.fillq:
                E.fillq.popleft()()
            elif tk == 15 and len(E.fillq) > 2:
                E.fillq.popleft()()
        pv(14, pipe.pop(0))
        pv(15, pipe.pop(0))
    else:
        pts = [st_exp(tk) for tk in range(16)]
        mid_cb()
        for tk in range(16):
            pv(tk, pts[tk])

    for h in range(2):
        # softmax sums (PSUM row 64) -> sums tile partition 0
        nc.vector.tensor_copy(sums_h[h][0:1, :], ctx2[h][64:65, :])
        # ctx drain with bf16 cast (h1 shifts base 0 -> 64)
        nc.vector.tensor_copy(
            E.ctxT[h * 64:(h + 1) * 64, tq0:tq0 + 512], ctx2[h][0:64, :])
    return (tq0, sums_h)


def _norm(E, tq0, sums_h):
    """Normalization + V-bias for one 512-token group (DVE/gpsimd only)."""
    nc = E.nc
    bcast = E.bc_pool.tile([128, 512], FP32, tag="bcast")
    bcb = E.bc_pool.tile([128, 512], FP32, tag="bcb")
    nc.gpsimd.partition_broadcast(bcast[0:64, :], sums_h[0][0:1, :])
    nc.gpsimd.partition_broadcast(bcb[0:64, :], sums_h[1][0:1, :])
    nc.vector.tensor_copy(bcast[64:128, :], bcb[0:64, :])
    recipb = E.bc_pool.tile([128, 512], FP32, tag="recipb")
    nc.vector.reciprocal_approx_fast(recipb[:], bcast[:])
    nc.vector.tensor_mul(E.ctxT[:, tq0:tq0 + 512], E.ctxT[:, tq0:tq0 + 512], recipb[:])
    nc.vector.tensor_scalar_add(
        E.ctxT[:, tq0:tq0 + 512], E.ctxT[:, tq0:tq0 + 512], E.bv_sb[:])


def _outproj_half(E, tq0, half2, out):
    """Output projection for one 256-token half of a 512-token group."""
    nc = E.nc
    for tc4 in (2 * half2, 2 * half2 + 1):
        t0 = tq0 + tc4 * 128
        for half in range(2):
            ops = E.po_ps.tile([128, 512], FP32, tag="po", name="ops")
            nc.tensor.matmul(
                ops[:],
                E.ctxT[:, t0:t0 + 128],
                E.wo_sb[:, half * 512:(half + 1) * 512],
                start=True,
                stop=True,
            )
            ostg = E.ostg_pool.tile([128, 512], FP16, tag="ostg")
            nc.vector.tensor_copy(ostg[:], ops[:])
            nc.sync.dma_start(
                out.ap()[t0:t0 + 128, half * 512:(half + 1) * 512], ostg[:])


# ---------------- host-side helpers ----------------

def core_inputs(q, k, v, Wq, bq_, Wk, bk_, Wv, bv_, Wo, core):
    """Build the per-core input map (numpy, host-side shard/layout prep)."""
    import ml_dtypes

    bf16 = ml_dtypes.bfloat16
    dsl = slice(core * PH, (core + 1) * PH)

    def warr(W):
        # [1024, 128] slice -> [128 part, KC*128] (kc-major per partition)
        w = W[:, dsl].reshape(KC, 128, PH).transpose(1, 0, 2).reshape(128, KC * PH)
        return np.ascontiguousarray(w).astype(bf16)

    return {
        "wq": warr(Wq),
        "wk": warr(Wk),
        "wv": warr(Wv),
        "wo": np.ascontiguousarray(Wo[dsl, :]).astype(bf16),
        "bq": np.ascontiguousarray(bq_[dsl]).reshape(PH, 1).astype(np.float32),
        "bk": np.ascontiguousarray(bk_[dsl]).reshape(PH, 1).astype(np.float32),
        "bv": np.ascontiguousarray(bv_[dsl]).reshape(PH, 1).astype(np.float32),
    }


def shared_inputs(q, k, v):
    import ml_dtypes

    bf16 = ml_dtypes.bfloat16
    qT_np = np.ascontiguousarray(q.reshape(N, D).T).astype(bf16)
    kT_np = np.ascontiguousarray(k.reshape(N, D).T).astype(bf16)
    vT_np = np.ascontiguousarray(v.reshape(N, D).T).astype(bf16)
    return {"qT": qT_np, "kT": kT_np, "vT": vT_np}


# ---------------- public entry point ----------------

_NC_CACHE = []


def _get_nc():
    if not _NC_CACHE:
        _NC_CACHE.append(build())
    return _NC_CACHE[0]


def kernel(q, k, v, Wq, bq, Wk, bk, Wv, bv, Wo, bo):
    from concourse import bass_utils

    q = np.asarray(q, np.float32)
    k = np.asarray(k, np.float32)
    v = np.asarray(v, np.float32)
    Wq, bq = np.asarray(Wq, np.float32), np.asarray(bq, np.float32)
    Wk, bk = np.asarray(Wk, np.float32), np.asarray(bk, np.float32)
    Wv, bv = np.asarray(Wv, np.float32), np.asarray(bv, np.float32)
    Wo, bo = np.asarray(Wo, np.float32), np.asarray(bo, np.float32)

    nc = _get_nc()
    shared = shared_inputs(q, k, v)
    in_maps = []
    for core in range(8):
        m = dict(shared)
        m.update(core_inputs(q, k, v, Wq, bq, Wk, bk, Wv, bv, Wo, core))
        in_maps.append(m)

    res = bass_utils.run_bass_kernel_spmd(nc, in_maps, core_ids=list(range(8)))

    acc = np.zeros((N, D), np.float64)
    for r in res.results:
        acc += r["out"].astype(np.float64)
    outp = (acc + bo.astype(np.float64)).astype(np.float32)
    return outp.reshape(B, T, D)



# revision 7
# speedup vs baseline: 1.0933x; 1.0933x over previous
"""Self-contained TRN2 Bass kernel for 16-head MHA (B=2, T=2048, D=1024),
head-parallel across 8 NeuronCores (2 heads per core).

kernel(**inputs) takes the FULL fp32 inputs of reference.setup_inputs() and
returns the FULL [2, 2048, 1024] fp32 output.  Host-side prep: q/k/v are
transposed to [1024, 4096] bf16 (shared by all cores); each core gets its
128-column slice of Wq/Wk/Wv (and 128-row slice of Wo) in bf16.  Each core
computes its two heads end-to-end (QKV projections, softmax attention with
row-group-packed score matmuls, ones-augmented PV for free softmax sums,
output projection) and DMAs a rank-128 partial of the output back; the host
sums the 8 partials and adds the output bias.

v2: PV-before-S^T emission (hides PV weight loads under the score stream),
fine-grained t-major DMA for batch-0 k/q/v so attention starts as soon as
the first q chunk is projected, piecewise V projection threaded into the
first attention group of each batch, and explicit per-group filler lists
that leave only the last group's normalization + output projection in the
tail.
"""

import numpy as np

import concourse.bass as bass
import concourse.mybir as mybir
import concourse.tile as tile
from concourse import bacc

FP32 = mybir.dt.float32
FP16 = mybir.dt.float16
BF16 = mybir.dt.bfloat16

D = 1024          # model dim
N = 4096          # B*T tokens
B = 2
T = 2048
PH = 128          # per-core projection dims (2 heads x 64)
DH = 64           # head dim
KC = 8            # contraction chunks (1024 / 128)
NTC = N // 128    # 32 token chunks of 128
SCALE = 0.125     # 1/sqrt(64)

ACT_EXP = mybir.ActivationFunctionType.Exp


def build(nc=None):
    if nc is None:
        nc = bacc.Bacc(
            "TRN2",
            target_bir_lowering=False,
            debug=False,
            enable_asserts=False,
            num_devices=8,
        )

    qT = nc.dram_tensor("qT", [D, N], BF16, kind="ExternalInput")
    kT = nc.dram_tensor("kT", [D, N], BF16, kind="ExternalInput")
    vT = nc.dram_tensor("vT", [D, N], BF16, kind="ExternalInput")
    # host pre-arranges W* into the [128, KC, 128] SBUF layout so the
    # weight DMA is one contiguous 2 KB-per-partition copy
    wq = nc.dram_tensor("wq", [128, KC * PH], BF16, kind="ExternalInput")
    wk = nc.dram_tensor("wk", [128, KC * PH], BF16, kind="ExternalInput")
    wv = nc.dram_tensor("wv", [128, KC * PH], BF16, kind="ExternalInput")
    wo = nc.dram_tensor("wo", [PH, D], BF16, kind="ExternalInput")
    bq = nc.dram_tensor("bq", [PH, 1], FP32, kind="ExternalInput")
    bk = nc.dram_tensor("bk", [PH, 1], FP32, kind="ExternalInput")
    bv = nc.dram_tensor("bv", [PH, 1], FP32, kind="ExternalInput")
    out = nc.dram_tensor("out", [N, D], FP16, kind="ExternalOutput")

    with tile.TileContext(nc) as tc:
        _emit(nc, tc, qT, kT, vT, wq, wk, wv, wo, bq, bk, bv, out)

    nc.compile()
    return nc


class _Ctx:
    pass


def _emit(nc, tc, qT, kT, vT, wq, wk, wv, wo, bq, bk, bv, out):
    from contextlib import ExitStack

    E = _Ctx()
    E.nc = nc

    ctxmgr = ExitStack()
    with ctxmgr:
        const_pool = ctxmgr.enter_context(tc.tile_pool(name="const", bufs=1))
        # fine-grained [128,512] input tiles (batch-0 k/q/v, t-major)
        E.xtf_pool = ctxmgr.enter_context(tc.tile_pool(name="xtf", bufs=52))
        # coarse [128,2048] input tiles (batch-1 k/q/v)
        E.xt_pool = ctxmgr.enter_context(tc.tile_pool(name="xt", bufs=10))
        big_pool = ctxmgr.enter_context(tc.tile_pool(name="big", bufs=1))
        E.pt_pool = ctxmgr.enter_context(tc.tile_pool(name="pt", bufs=7))
        E.bc_pool = ctxmgr.enter_context(tc.tile_pool(name="bc", bufs=1))
        E.vt_pool = ctxmgr.enter_context(tc.tile_pool(name="vt", bufs=2))
        E.ostg_pool = ctxmgr.enter_context(tc.tile_pool(name="ostg", bufs=5))
        # PSUM: proj/outproj/transpose 2 banks + st 4 + ctx 2 = 8
        E.po_ps = ctxmgr.enter_context(
            tc.tile_pool(name="po_ps", bufs=2, space="PSUM"))
        E.st_ps = ctxmgr.enter_context(
            tc.tile_pool(name="st_ps", bufs=2, space="PSUM"))
        E.ctx_ps = ctxmgr.enter_context(
            tc.tile_pool(name="ctx_ps", bufs=2, space="PSUM"))

        # --- weights / consts to SBUF ---
        wq_sb = const_pool.tile([128, KC, PH], BF16, tag="wq")
        wk_sb = const_pool.tile([128, KC, PH], BF16, tag="wk")
        wv_sb = const_pool.tile([128, KC, PH], BF16, tag="wv")
        E.wo_sb = const_pool.tile([128, D], BF16, tag="wo")
        bq_sb = const_pool.tile([128, 1], FP32, tag="bq")
        bk_sb = const_pool.tile([128, 1], FP32, tag="bk")
        E.bv_sb = const_pool.tile([128, 1], FP32, tag="bv")
        nc.sync.dma_start(wk_sb[:], wk.ap().rearrange("p (c m) -> p c m", c=KC))
        nc.sync.dma_start(bk_sb[:], bk.ap())

        # persistent activations
        E.qT_sb = big_pool.tile([128, N], BF16, tag="qTsb")
        E.kT_sb = big_pool.tile([128, N], BF16, tag="kTsb")
        # v_aug pair layout: [tok part, 32 tok chunks, 130]; per head h the
        # PV stationary operand is vp[:, chunk, 65h : 65h+65] = [v_h | ones]
        E.vp = big_pool.tile([128, NTC, 130], BF16, tag="vp")
        E.ctxT = big_pool.tile([128, N], BF16, tag="ctxT")

        nc.gpsimd.memset(E.vp[:, :, 64], 1.0)
        nc.gpsimd.memset(E.vp[:, :, 129], 1.0)

        E.identity = const_pool.tile([128, 128], BF16, tag="ident")
        from concourse.masks import make_identity
        make_identity(nc, E.identity[:])

        def dma_fine(nm, xdram, b, t):
            """One t-major fine tile set: [8 x [128,512]] so a proj chunk
            can start as soon as its own 1MB lands."""
            lst = []
            for kc in range(KC):
                xt = E.xtf_pool.tile(
                    [128, 512], BF16, tag="xtf", bufs=52,
                    name=f"xf_{nm}{b}{t}{kc}")
                nc.sync.dma_start(
                    xt[:],
                    xdram.ap()[kc * 128:(kc + 1) * 128,
                               b * T + t * 512: b * T + (t + 1) * 512])
                lst.append(xt)
            return lst

        def dma_coarse(nm, xdram, b, tag, bufs):
            lst = []
            for kc in range(KC):
                xt = E.xt_pool.tile(
                    [128, T], BF16, tag=tag, bufs=bufs,
                    name=f"xt_{nm}{b}{kc}")
                nc.sync.dma_start(
                    xt[:], xdram.ap()[kc * 128:(kc + 1) * 128, b * T:(b + 1) * T])
                lst.append(xt)
            return lst

        def group(b, tqc, vcb=None, fillers=()):
            return _attention_group(E, b, tqc, vcb, list(fillers))

        def pchunk_f(xts, wsb, bias_sb, dstT, b, t):
            # fine tile set: columns 0:512 of the set are exactly chunk t
            return lambda: _proj_chunk(E, xts, wsb, bias_sb, dstT, b * T, t,
                                       tile_t0=t)

        def pchunk(xts, wsb, bias_sb, dstT, b, t):
            return lambda: _proj_chunk(E, xts, wsb, bias_sb, dstT, b * T, t)

        def unorm(pend, half2):
            # half2==0 also runs the (cheap, DVE-side) normalization
            tq0, sums = pend

            def run():
                if half2 == 0:
                    _norm(E, tq0, sums)
                _outproj_half(E, tq0, half2, out)
            return run

        def make_vcb(getx, b):
            """Piecewise V projection: chunk j projects tokens [512j,512j+512)
            of batch b and transposes them into vp, so PV(tk) only needs
            chunk tk//4 done.  getx(j, kc) returns the [128,512] input AP."""
            def vcb(j):
                vt = E.vt_pool.tile([128, 512], BF16, tag="vt", bufs=2,
                                    name=f"vt{b}{j}")
                ps = E.po_ps.tile([128, 512], FP32, tag="po", name="vps")
                for kc in range(KC):
                    nc.tensor.matmul(
                        ps[:], wv_sb[:, kc, :], getx(j, kc),
                        start=(kc == 0), stop=(kc == KC - 1))
                nc.vector.tensor_copy(vt[:], ps[:])
                for tloc in range(4):
                    tcid = b * 16 + j * 4 + tloc
                    tr = E.po_ps.tile([128, 128], BF16, tag="po",
                                      name=f"tr{tcid}")
                    nc.tensor.transpose(
                        tr[:], vt[:, tloc * 128:(tloc + 1) * 128], E.identity[:])
                    nc.vector.tensor_copy(E.vp[:, tcid, 0:64], tr[:, 0:64])
                    nc.vector.tensor_copy(E.vp[:, tcid, 65:129], tr[:, 64:128])
            return vcb

        # ---- priming: batch-0 DMAs fine-grained t-major, ordered so each
        # consumer's megabyte lands just in time: k (proj now), q chunk 0
        # (first group), then v chunks (vcb slots inside group 0), then the
        # remaining q chunks (fillers in group 1).
        xk0 = [dma_fine("k", kT, 0, t) for t in range(4)]
        nc.sync.dma_start(wq_sb[:], wq.ap().rearrange("p (c m) -> p c m", c=KC))
        nc.sync.dma_start(bq_sb[:], bq.ap())
        xq0_0 = dma_fine("q", qT, 0, 0)
        nc.sync.dma_start(wv_sb[:], wv.ap().rearrange("p (c m) -> p c m", c=KC))
        nc.sync.dma_start(E.wo_sb[:], wo.ap())
        nc.sync.dma_start(E.bv_sb[:], bv.ap())
        xv0 = [dma_fine("v", vT, 0, t) for t in range(4)]
        xq0r = {t: dma_fine("q", qT, 0, t) for t in (1, 2, 3)}

        for t in range(4):
            _proj_chunk(E, xk0[t], wk_sb, bk_sb, E.kT_sb, 0, t, tile_t0=t,
                        drain_act=True)
        _proj_chunk(E, xq0_0, wq_sb, bq_sb, E.qT_sb, 0, 0, tile_t0=0,
                    drain_act=True)

        g00 = group(0, 0, vcb=make_vcb(lambda j, kc: xv0[j][kc][:, 0:512], 0))
        xk1 = dma_coarse("k", kT, 1, "xt", 10)
        _proj_chunk(E, xq0r[1], wq_sb, bq_sb, E.qT_sb, 0, 1, tile_t0=1)
        g01 = group(0, 1, fillers=[
            pchunk_f(xq0r[2], wq_sb, bq_sb, E.qT_sb, 0, 2),
            pchunk_f(xq0r[3], wq_sb, bq_sb, E.qT_sb, 0, 3),
            pchunk(xk1, wk_sb, bk_sb, E.kT_sb, 1, 0),
            pchunk(xk1, wk_sb, bk_sb, E.kT_sb, 1, 1),
        ])
        xq1 = dma_coarse("q", qT, 1, "xt", 10)
        g02 = group(0, 2, fillers=[
            pchunk(xk1, wk_sb, bk_sb, E.kT_sb, 1, 2),
            pchunk(xk1, wk_sb, bk_sb, E.kT_sb, 1, 3),
            unorm(g00, 0),
            unorm(g00, 1),
        ])
        g03 = group(0, 3, fillers=[
            pchunk(xq1, wq_sb, bq_sb, E.qT_sb, 1, 0),
            unorm(g01, 0),
            unorm(g01, 1),
        ])
        xv1 = dma_coarse("v", vT, 1, "xtv", 8)
        g10 = group(1, 0, vcb=make_vcb(
            lambda j, kc: xv1[kc][:, j * 512:(j + 1) * 512], 1),
            fillers=[
                pchunk(xq1, wq_sb, bq_sb, E.qT_sb, 1, 1),
                unorm(g02, 0),
            ])
        g11 = group(1, 1, fillers=[
            pchunk(xq1, wq_sb, bq_sb, E.qT_sb, 1, 2),
            pchunk(xq1, wq_sb, bq_sb, E.qT_sb, 1, 3),
            unorm(g02, 1),
            unorm(g03, 0),
        ])
        g12 = group(1, 2, fillers=[
            unorm(g03, 1),
            unorm(g10, 0),
            unorm(g10, 1),
            unorm(g11, 0),
        ])
        g13 = group(1, 3, fillers=[
            unorm(g11, 1),
            unorm(g12, 0),
            unorm(g12, 1),
        ])
        unorm(g13, 0)()
        unorm(g13, 1)()


def _proj_chunk(E, xts, wsb, bias_sb, dstT, btok, t, tile_t0=0,
                drain_act=False):
    """One 512-token projection chunk: accumulate 8 kc matmuls, drain.
    xts tiles start at token column tile_t0*512 of this batch."""
    nc = E.nc
    ps = E.po_ps.tile([128, 512], FP32, tag="po", name="ps")
    c0 = (t - tile_t0) * 512
    for kc in range(KC):
        nc.tensor.matmul(
            ps[:],
            wsb[:, kc, :],
            xts[kc][:, c0:c0 + 512],
            start=(kc == 0),
            stop=(kc == KC - 1),
        )
    dst = dstT[:, btok + t * 512: btok + (t + 1) * 512]
    if drain_act:
        # ScalarE drain (idle during priming); Identity has a free
        # per-partition bias add
        if bias_sb is not None:
            nc.scalar.activation(
                dst, ps[:], mybir.ActivationFunctionType.Identity, bias=bias_sb[:])
        else:
            nc.scalar.activation(dst, ps[:], mybir.ActivationFunctionType.Identity)
    elif bias_sb is not None:
        nc.vector.tensor_scalar_add(dst, ps[:], bias_sb[:])
    else:
        nc.vector.tensor_copy(dst, ps[:])


def _attention_group(E, b, tqc, vcb, fillers):
    """S^T/exp/PV + sums & ctx drains for one 512-token group (both heads).

    The two heads' S^T matmuls are row-group packed: head h's K=64
    contraction occupies array rows 64h..64h+63, so the pair runs
    concurrently on the PE.  st is one flat [128, 1024] PSUM tile (2 banks,
    head h in columns 512h..512h+512) so the exp is a single contiguous
    2-D ACTIVATE over 1024 elements per partition.

    PV(tk-3) is emitted BEFORE S^T(tk): each matmul's weight load then
    hides under the previous matmul's moving stream (the PV loads are
    128 rows and do not fit behind their own pair's stream otherwise).
    Filler units (deferred projections, norm+output-projection halves)
    are paced into the exp-bound slack at fixed tk slots.

    With vcb set (first group of each batch), vcb(j) is emitted at
    tk = 4j+3 to project+transpose V tokens [512j, 512j+512), and PV
    trails by 4 so PV(tk) always finds its vp chunk ready.
    """
    nc = E.nc
    btok = b * T
    tq0 = btok + tqc * 512

    # both heads' softmax sums side by side in the free dim (partition
    # offsets must be 32-aligned, so two partitions is not an option)
    sums = E.bc_pool.tile([1, 1024], FP32, tag="sums", bufs=4,
                          name=f"sums{b}{tqc}")
    ctx2 = [
        E.ctx_ps.tile([65, 512], FP32, tag="ctx", name=f"ctx{h}")
        for h in range(2)
    ]

    def st_exp(tk):
        st = E.st_ps.tile([128, 1024], FP32, tag="st", name="st")
        for h in range(2):
            nc.tensor.matmul(
                st[:, h * 512:(h + 1) * 512],
                E.kT_sb[h * 64:(h + 1) * 64,
                        btok + tk * 128: btok + (tk + 1) * 128],
                E.qT_sb[h * 64:(h + 1) * 64, tq0:tq0 + 512],
                start=True,
                stop=True,
            )
        pt = E.pt_pool.tile([128, 1024], BF16, tag="pt", name="pt")
        nc.scalar.activation(pt[:], st[:], ACT_EXP, scale=SCALE)
        return pt

    def pv(tk, pt):
        for h in range(2):
            nc.tensor.matmul(
                ctx2[h][:],
                E.vp[:, b * 16 + tk, h * 65:(h + 1) * 65],
                pt[:, h * 512:(h + 1) * 512],
                start=(tk == 0),
                stop=(tk == 15),
            )

    fill = list(fillers)
    pipe = []
    if vcb is None:
        for tk in range(16):
            if len(pipe) >= 3:
                pv(tk - 3, pipe.pop(0))
            pipe.append(st_exp(tk))
            if tk in (3, 7, 11, 15) and fill:
                fill.pop(0)()
        for i, pt in enumerate(pipe):
            pv(13 + i, pt)
    else:
        for tk in range(16):
            if len(pipe) >= 4:
                pv(tk - 4, pipe.pop(0))
            pipe.append(st_exp(tk))
            if tk % 4 == 3:
                vcb(tk // 4)
            elif tk in (5, 13) and fill:
                fill.pop(0)()
        for i, pt in enumerate(pipe):
            pv(12 + i, pt)
    assert not fill

    for h in range(2):
        # softmax sums (PSUM row 64) -> sums tile columns 512h..512h+512
        nc.vector.tensor_copy(
            sums[0:1, h * 512:(h + 1) * 512], ctx2[h][64:65, :])
        # ctx drain with bf16 cast (h1 shifts base 0 -> 64)
        nc.vector.tensor_copy(
            E.ctxT[h * 64:(h + 1) * 64, tq0:tq0 + 512], ctx2[h][0:64, :])
    return (tq0, sums)


def _norm(E, tq0, sums):
    """Normalization + V-bias for one 512-token group (DVE/gpsimd only)."""
    nc = E.nc
    bcast = E.bc_pool.tile([128, 512], FP32, tag="bcast", bufs=1)
    bcb = E.bc_pool.tile([128, 512], FP32, tag="bcb", bufs=1)
    nc.gpsimd.partition_broadcast(bcast[0:64, :], sums[0:1, 0:512])
    nc.gpsimd.partition_broadcast(bcb[0:64, :], sums[0:1, 512:1024])
    nc.vector.tensor_copy(bcast[64:128, :], bcb[0:64, :])
    recipb = E.bc_pool.tile([128, 512], FP32, tag="recipb", bufs=1)
    nc.vector.reciprocal_approx_fast(recipb[:], bcast[:])
    nc.vector.tensor_mul(E.ctxT[:, tq0:tq0 + 512], E.ctxT[:, tq0:tq0 + 512], recipb[:])
    nc.vector.tensor_scalar_add(
        E.ctxT[:, tq0:tq0 + 512], E.ctxT[:, tq0:tq0 + 512], E.bv_sb[:])


def _outproj_half(E, tq0, half2, out):
    """Output projection for one 256-token half of a 512-token group."""
    nc = E.nc
    for tc4 in (2 * half2, 2 * half2 + 1):
        t0 = tq0 + tc4 * 128
        for half in range(2):
            ops = E.po_ps.tile([128, 512], FP32, tag="po", name="ops")
            nc.tensor.matmul(
                ops[:],
                E.ctxT[:, t0:t0 + 128],
                E.wo_sb[:, half * 512:(half + 1) * 512],
                start=True,
                stop=True,
            )
            ostg = E.ostg_pool.tile([128, 512], FP16, tag="ostg")
            nc.vector.tensor_copy(ostg[:], ops[:])
            nc.sync.dma_start(
                out.ap()[t0:t0 + 128, half * 512:(half + 1) * 512], ostg[:])


# ---------------- host-side helpers ----------------

def core_inputs(q, k, v, Wq, bq_, Wk, bk_, Wv, bv_, Wo, core):
    """Build the per-core input map (numpy, host-side shard/layout prep)."""
    import ml_dtypes

    bf16 = ml_dtypes.bfloat16
    dsl = slice(core * PH, (core + 1) * PH)

    def warr(W):
        # [1024, 128] slice -> [128 part, KC*128] (kc-major per partition)
        w = W[:, dsl].reshape(KC, 128, PH).transpose(1, 0, 2).reshape(128, KC * PH)
        return np.ascontiguousarray(w).astype(bf16)

    return {
        "wq": warr(Wq),
        "wk": warr(Wk),
        "wv": warr(Wv),
        "wo": np.ascontiguousarray(Wo[dsl, :]).astype(bf16),
        "bq": np.ascontiguousarray(bq_[dsl]).reshape(PH, 1).astype(np.float32),
        "bk": np.ascontiguousarray(bk_[dsl]).reshape(PH, 1).astype(np.float32),
        "bv": np.ascontiguousarray(bv_[dsl]).reshape(PH, 1).astype(np.float32),
    }


def shared_inputs(q, k, v):
    import ml_dtypes

    bf16 = ml_dtypes.bfloat16
    qT_np = np.ascontiguousarray(q.reshape(N, D).T).astype(bf16)
    kT_np = np.ascontiguousarray(k.reshape(N, D).T).astype(bf16)
    vT_np = np.ascontiguousarray(v.reshape(N, D).T).astype(bf16)
    return {"qT": qT_np, "kT": kT_np, "vT": vT_np}


# ---------------- public entry point ----------------

_NC_CACHE = []


def _get_nc():
    if not _NC_CACHE:
        _NC_CACHE.append(build())
    return _NC_CACHE[0]


def kernel(q, k, v, Wq, bq, Wk, bk, Wv, bv, Wo, bo):
    from concourse import bass_utils

    q = np.asarray(q, np.float32)
    k = np.asarray(k, np.float32)
    v = np.asarray(v, np.float32)
    Wq, bq = np.asarray(Wq, np.float32), np.asarray(bq, np.float32)
    Wk, bk = np.asarray(Wk, np.float32), np.asarray(bk, np.float32)
    Wv, bv = np.asarray(Wv, np.float32), np.asarray(bv, np.float32)
    Wo, bo = np.asarray(Wo, np.float32), np.asarray(bo, np.float32)

    nc = _get_nc()
    shared = shared_inputs(q, k, v)
    in_maps = []
    for core in range(8):
        m = dict(shared)
        m.update(core_inputs(q, k, v, Wq, bq, Wk, bk, Wv, bv, Wo, core))
        in_maps.append(m)

    res = bass_utils.run_bass_kernel_spmd(nc, in_maps, core_ids=list(range(8)))

    acc = np.zeros((N, D), np.float64)
    for r in res.results:
        acc += r["out"].astype(np.float64)
    outp = (acc + bo.astype(np.float64)).astype(np.float32)
    return outp.reshape(B, T, D)


# revision 20
# speedup vs baseline: 1.1699x; 1.0701x over previous
"""Self-contained TRN2 Bass kernel for 16-head MHA (B=2, T=2048, D=1024),
head-parallel across 8 NeuronCores (2 heads per core).

kernel(**inputs) takes the FULL fp32 inputs of reference.setup_inputs() and
returns the FULL [2, 2048, 1024] fp32 output.  Host-side prep: q/k/v are
transposed to [1024, 4096] bf16 (shared by all cores); each core gets its
128-column slice of Wq/Wk/Wv (and 128-row slice of Wo) in bf16.  Each core
computes its two heads end-to-end (QKV projections, softmax attention with
row-group-packed score matmuls, ones-augmented PV for free softmax sums,
output projection) and DMAs a rank-128 partial of the output back; the host
sums the 8 partials and adds the output bias.

v2: PV-before-S^T emission (hides PV weight loads under the score stream),
fine-grained t-major DMA for batch-0 k/q/v so attention starts as soon as
the first q chunk is projected, piecewise V projection threaded into the
first attention group of each batch, and explicit per-group filler lists
that leave only the last group's normalization + output projection in the
tail.
"""

import numpy as np

import concourse.bass as bass
import concourse.mybir as mybir
import concourse.tile as tile
from concourse import bacc

FP32 = mybir.dt.float32
FP16 = mybir.dt.float16
BF16 = mybir.dt.bfloat16

D = 1024          # model dim
N = 4096          # B*T tokens
B = 2
T = 2048
PH = 128          # per-core projection dims (2 heads x 64)
DH = 64           # head dim
KC = 8            # contraction chunks (1024 / 128)
NTC = N // 128    # 32 token chunks of 128
SCALE = 0.125     # 1/sqrt(64)

ACT_EXP = mybir.ActivationFunctionType.Exp


def build(nc=None):
    if nc is None:
        nc = bacc.Bacc(
            "TRN2",
            target_bir_lowering=False,
            debug=False,
            enable_asserts=False,
            num_devices=8,
        )

    qT = nc.dram_tensor("qT", [D, N], BF16, kind="ExternalInput")
    kT = nc.dram_tensor("kT", [D, N], BF16, kind="ExternalInput")
    vT = nc.dram_tensor("vT", [D, N], BF16, kind="ExternalInput")
    # host pre-arranges W* into the [128, KC, 128] SBUF layout so the
    # weight DMA is one contiguous 2 KB-per-partition copy
    wq = nc.dram_tensor("wq", [128, KC * PH], BF16, kind="ExternalInput")
    wk = nc.dram_tensor("wk", [128, KC * PH], BF16, kind="ExternalInput")
    wv = nc.dram_tensor("wv", [128, KC * PH], BF16, kind="ExternalInput")
    wo = nc.dram_tensor("wo", [PH, D], BF16, kind="ExternalInput")
    bq = nc.dram_tensor("bq", [PH, 1], FP32, kind="ExternalInput")
    bk = nc.dram_tensor("bk", [PH, 1], FP32, kind="ExternalInput")
    bv = nc.dram_tensor("bv", [PH, 1], FP32, kind="ExternalInput")
    out = nc.dram_tensor("out", [N, D], FP16, kind="ExternalOutput")

    with tile.TileContext(nc) as tc:
        _emit(nc, tc, qT, kT, vT, wq, wk, wv, wo, bq, bk, bv, out)

    nc.compile()
    return nc


class _Ctx:
    pass


def _emit(nc, tc, qT, kT, vT, wq, wk, wv, wo, bq, bk, bv, out):
    from contextlib import ExitStack

    E = _Ctx()
    E.nc = nc

    ctxmgr = ExitStack()
    with ctxmgr:
        const_pool = ctxmgr.enter_context(tc.tile_pool(name="const", bufs=1))
        # batch-0 [128, KC, 512] t-major mega tiles: ONE dma_start each
        # (the Sync engine issues DMA descriptors at ~610ns per dma_start,
        # so the priming path must use as few dma_starts as possible)
        E.xm_pool = ctxmgr.enter_context(tc.tile_pool(name="xm", bufs=6))
        # coarse [128,2048] input tiles (batch-1 k/q/v)
        E.xt_pool = ctxmgr.enter_context(tc.tile_pool(name="xt", bufs=9))
        big_pool = ctxmgr.enter_context(tc.tile_pool(name="big", bufs=1))
        E.pt_pool = ctxmgr.enter_context(tc.tile_pool(name="pt", bufs=6))
        E.bc_pool = ctxmgr.enter_context(tc.tile_pool(name="bc", bufs=1))
        E.vt_pool = ctxmgr.enter_context(tc.tile_pool(name="vt", bufs=2))
        E.ostg_pool = ctxmgr.enter_context(tc.tile_pool(name="ostg", bufs=4))
        # PSUM: proj/outproj/transpose 2 banks + st 4 + ctx 2 = 8
        E.po_ps = ctxmgr.enter_context(
            tc.tile_pool(name="po_ps", bufs=2, space="PSUM"))
        E.st_ps = ctxmgr.enter_context(
            tc.tile_pool(name="st_ps", bufs=2, space="PSUM"))
        E.ctx_ps = ctxmgr.enter_context(
            tc.tile_pool(name="ctx_ps", bufs=2, space="PSUM"))

        # --- weights / consts to SBUF ---
        wq_sb = const_pool.tile([128, KC, PH], BF16, tag="wq")
        wk_sb = const_pool.tile([128, KC, PH], BF16, tag="wk")
        wv_sb = const_pool.tile([128, KC, PH], BF16, tag="wv")
        E.wo_sb = const_pool.tile([128, D], BF16, tag="wo")
        bq_sb = const_pool.tile([128, 1], FP32, tag="bq")
        bk_sb = const_pool.tile([128, 1], FP32, tag="bk")
        E.bv_sb = const_pool.tile([128, 1], FP32, tag="bv")
        nc.sync.dma_start(wk_sb[:], wk.ap().rearrange("p (c m) -> p c m", c=KC))
        nc.sync.dma_start(bk_sb[:], bk.ap())

        # persistent activations
        E.qT_sb = big_pool.tile([128, N], BF16, tag="qTsb")
        E.kT_sb = big_pool.tile([128, N], BF16, tag="kTsb")
        # v_aug pair layout: [tok part, 32 tok chunks, 130]; per head h the
        # PV stationary operand is vp[:, chunk, 65h : 65h+65] = [v_h | ones]
        E.vp = big_pool.tile([128, NTC, 130], BF16, tag="vp")
        E.ctxT = big_pool.tile([128, N], BF16, tag="ctxT")

        nc.gpsimd.memset(E.vp[:, :, 64], 1.0)
        nc.gpsimd.memset(E.vp[:, :, 129], 1.0)

        E.identity = const_pool.tile([128, 128], BF16, tag="ident")
        from concourse.masks import make_identity
        make_identity(nc, E.identity[:])

        # masks for the tail-normalization PE broadcast (all partition-0):
        # cols 0:128 select out partitions 0:64 (head 0), cols 128:256
        # select out partitions 64:128 (head 1)
        E.nmask = const_pool.tile([128, 256], FP32, tag="nmask")
        nc.gpsimd.memset(E.nmask[:], 0.0)
        nc.gpsimd.memset(E.nmask[0:1, 0:64], 1.0)
        nc.gpsimd.memset(E.nmask[0:1, 192:256], 1.0)

        def dma_fine(nm, xdram, b, t):
            """One t-major [128, KC, 512] mega tile (a single dma_start:
            partition p, chunk c <- dram row c*128+p) so a proj chunk can
            start as soon as its own 1MB lands.  Returns per-kc APs."""
            xm = E.xm_pool.tile(
                [128, KC, 512], BF16, tag="xm", bufs=6,
                name=f"xm_{nm}{b}{t}")
            nc.sync.dma_start(
                xm[:],
                xdram.ap()[:, b * T + t * 512: b * T + (t + 1) * 512]
                .rearrange("(c p) m -> p c m", p=128))
            return [xm[:, kc, :] for kc in range(KC)]

        def dma_coarse(nm, xdram, b, tag, bufs):
            lst = []
            for kc in range(KC):
                xt = E.xt_pool.tile(
                    [128, T], BF16, tag=tag, bufs=bufs,
                    name=f"xt_{nm}{b}{kc}")
                nc.sync.dma_start(
                    xt[:], xdram.ap()[kc * 128:(kc + 1) * 128, b * T:(b + 1) * T])
                lst.append(xt)
            return lst

        def group(b, tqc, vcb=None, fillers=()):
            return _attention_group(E, b, tqc, vcb, list(fillers))

        def pchunk_f(xts, wsb, bias_sb, dstT, b, t):
            # fine tile set: columns 0:512 of the set are exactly chunk t
            return lambda: _proj_chunk(E, xts, wsb, bias_sb, dstT, b * T, t,
                                       tile_t0=t)

        def pchunk(xts, wsb, bias_sb, dstT, b, t):
            return lambda: _proj_chunk(E, xts, wsb, bias_sb, dstT, b * T, t)

        def unorm(pend, half2, tail=False):
            # half2==0 also runs the (cheap, DVE-side) normalization
            tq0, sums = pend

            def run():
                if half2 == 0:
                    if tail:
                        _norm_tail(E, tq0, sums)
                    else:
                        _norm(E, tq0, sums)
                _outproj_half(E, tq0, half2, out)
            return run

        def make_vcb(getx, b):
            """Piecewise V projection: chunk j projects tokens [512j,512j+512)
            of batch b and transposes them into vp, so PV(tk) only needs
            chunk tk//4 done.  getx(j, kc) returns the [128,512] input AP."""
            def vcb(j):
                vt = E.vt_pool.tile([128, 512], BF16, tag="vt", bufs=2,
                                    name=f"vt{b}{j}")
                ps = E.po_ps.tile([128, 512], FP32, tag="po", name="vps")
                for kc in range(KC):
                    nc.tensor.matmul(
                        ps[:], wv_sb[:, kc, :], getx(j, kc),
                        start=(kc == 0), stop=(kc == KC - 1))
                nc.vector.tensor_copy(vt[:], ps[:])
                for tloc in range(4):
                    tcid = b * 16 + j * 4 + tloc
                    tr = E.po_ps.tile([128, 128], BF16, tag="po",
                                      name=f"tr{tcid}")
                    nc.tensor.transpose(
                        tr[:], vt[:, tloc * 128:(tloc + 1) * 128], E.identity[:])
                    nc.vector.tensor_copy(E.vp[:, tcid, 0:64], tr[:, 0:64])
                    nc.vector.tensor_copy(E.vp[:, tcid, 65:129], tr[:, 64:128])
            return vcb

        # ---- priming: batch-0 DMAs fine-grained t-major, ordered so each
        # consumer's megabyte lands just in time: k (proj now), q chunk 0
        # (first group), then v chunks (vcb slots inside group 0), then the
        # remaining q chunks (fillers in group 1).
        xk0 = [dma_fine("k", kT, 0, t) for t in range(4)]
        nc.sync.dma_start(wq_sb[:], wq.ap().rearrange("p (c m) -> p c m", c=KC))
        nc.sync.dma_start(bq_sb[:], bq.ap())
        xq0_0 = dma_fine("q", qT, 0, 0)
        nc.sync.dma_start(wv_sb[:], wv.ap().rearrange("p (c m) -> p c m", c=KC))
        nc.sync.dma_start(E.wo_sb[:], wo.ap())
        nc.sync.dma_start(E.bv_sb[:], bv.ap())
        xv0 = [dma_fine("v", vT, 0, t) for t in range(4)]
        xq0r = {t: dma_fine("q", qT, 0, t) for t in (1, 2, 3)}

        for t in range(4):
            _proj_chunk(E, xk0[t], wk_sb, bk_sb, E.kT_sb, 0, t, tile_t0=t,
                        drain_act=True)
        _proj_chunk(E, xq0_0, wq_sb, bq_sb, E.qT_sb, 0, 0, tile_t0=0,
                    drain_act=True)

        g00 = group(0, 0, vcb=make_vcb(lambda j, kc: xv0[j][kc], 0))
        xk1 = dma_coarse("k", kT, 1, "xt", 9)
        _proj_chunk(E, xq0r[1], wq_sb, bq_sb, E.qT_sb, 0, 1, tile_t0=1)
        g01 = group(0, 1, fillers=[
            pchunk_f(xq0r[2], wq_sb, bq_sb, E.qT_sb, 0, 2),
            pchunk_f(xq0r[3], wq_sb, bq_sb, E.qT_sb, 0, 3),
            pchunk(xk1, wk_sb, bk_sb, E.kT_sb, 1, 0),
            pchunk(xk1, wk_sb, bk_sb, E.kT_sb, 1, 1),
        ])
        xq1 = dma_coarse("q", qT, 1, "xt", 9)
        g02 = group(0, 2, fillers=[
            pchunk(xk1, wk_sb, bk_sb, E.kT_sb, 1, 2),
            pchunk(xk1, wk_sb, bk_sb, E.kT_sb, 1, 3),
            unorm(g00, 0),
            unorm(g00, 1),
        ])
        g03 = group(0, 3, fillers=[
            pchunk(xq1, wq_sb, bq_sb, E.qT_sb, 1, 0),
            unorm(g01, 0),
            unorm(g01, 1),
        ])
        xv1 = dma_coarse("v", vT, 1, "xtv", 8)
        g10 = group(1, 0, vcb=make_vcb(
            lambda j, kc: xv1[kc][:, j * 512:(j + 1) * 512], 1),
            fillers=[
                pchunk(xq1, wq_sb, bq_sb, E.qT_sb, 1, 1),
                unorm(g02, 0),
            ])
        g11 = group(1, 1, fillers=[
            pchunk(xq1, wq_sb, bq_sb, E.qT_sb, 1, 2),
            pchunk(xq1, wq_sb, bq_sb, E.qT_sb, 1, 3),
            unorm(g02, 1),
            unorm(g03, 0),
        ])
        g12 = group(1, 2, fillers=[
            unorm(g03, 1),
            unorm(g10, 0),
            unorm(g10, 1),
            unorm(g11, 0),
        ])
        g13 = group(1, 3, fillers=[
            unorm(g11, 1),
            unorm(g12, 0),
            unorm(g12, 1),
        ])
        unorm(g13, 0, tail=True)()
        unorm(g13, 1)()


def _proj_chunk(E, xts, wsb, bias_sb, dstT, btok, t, tile_t0=0,
                drain_act=False):
    """One 512-token projection chunk: accumulate 8 kc matmuls, drain.
    xts tiles start at token column tile_t0*512 of this batch."""
    nc = E.nc
    ps = E.po_ps.tile([128, 512], FP32, tag="po", name="ps")
    c0 = (t - tile_t0) * 512
    for kc in range(KC):
        nc.tensor.matmul(
            ps[:],
            wsb[:, kc, :],
            xts[kc][:, c0:c0 + 512],
            start=(kc == 0),
            stop=(kc == KC - 1),
        )
    dst = dstT[:, btok + t * 512: btok + (t + 1) * 512]
    if drain_act:
        # ScalarE drain (idle during priming); Identity has a free
        # per-partition bias add
        if bias_sb is not None:
            nc.scalar.activation(
                dst, ps[:], mybir.ActivationFunctionType.Identity, bias=bias_sb[:])
        else:
            nc.scalar.activation(dst, ps[:], mybir.ActivationFunctionType.Identity)
    elif bias_sb is not None:
        nc.vector.tensor_scalar_add(dst, ps[:], bias_sb[:])
    else:
        nc.vector.tensor_copy(dst, ps[:])


def _attention_group(E, b, tqc, vcb, fillers):
    """S^T/exp/PV + sums & ctx drains for one 512-token group (both heads).

    The two heads' S^T matmuls are row-group packed: head h's K=64
    contraction occupies array rows 64h..64h+63, so the pair runs
    concurrently on the PE.  st is one flat [128, 1024] PSUM tile (2 banks,
    head h in columns 512h..512h+512) so the exp is a single contiguous
    2-D ACTIVATE over 1024 elements per partition.

    PV(tk-3) is emitted BEFORE S^T(tk): each matmul's weight load then
    hides under the previous matmul's moving stream (the PV loads are
    128 rows and do not fit behind their own pair's stream otherwise).
    Filler units (deferred projections, norm+output-projection halves)
    are paced into the exp-bound slack at fixed tk slots.

    With vcb set (first group of each batch), vcb(j) is emitted at
    tk = 4j+3 to project+transpose V tokens [512j, 512j+512), and PV
    trails by 4 so PV(tk) always finds its vp chunk ready.
    """
    nc = E.nc
    btok = b * T
    tq0 = btok + tqc * 512

    # both heads' softmax sums side by side in the free dim (partition
    # offsets must be 32-aligned, so two partitions is not an option)
    sums = E.bc_pool.tile([1, 1024], FP32, tag="sums", bufs=4,
                          name=f"sums{b}{tqc}")
    ctx2 = [
        E.ctx_ps.tile([65, 512], FP32, tag="ctx", name=f"ctx{h}")
        for h in range(2)
    ]

    def st_exp(tk):
        st = E.st_ps.tile([128, 1024], FP32, tag="st", name="st")
        for h in range(2):
            nc.tensor.matmul(
                st[:, h * 512:(h + 1) * 512],
                E.kT_sb[h * 64:(h + 1) * 64,
                        btok + tk * 128: btok + (tk + 1) * 128],
                E.qT_sb[h * 64:(h + 1) * 64, tq0:tq0 + 512],
                start=True,
                stop=True,
            )
        pt = E.pt_pool.tile([128, 1024], BF16, tag="pt", name="pt")
        nc.scalar.activation(pt[:], st[:], ACT_EXP, scale=SCALE)
        return pt

    def pv(tk, pt):
        for h in range(2):
            nc.tensor.matmul(
                ctx2[h][:],
                E.vp[:, b * 16 + tk, h * 65:(h + 1) * 65],
                pt[:, h * 512:(h + 1) * 512],
                start=(tk == 0),
                stop=(tk == 15),
            )

    fill = list(fillers)
    pipe = []
    if vcb is None:
        for tk in range(16):
            if len(pipe) >= 3:
                pv(tk - 3, pipe.pop(0))
            pipe.append(st_exp(tk))
            if tk in (3, 7, 11, 15) and fill:
                fill.pop(0)()
        for i, pt in enumerate(pipe):
            pv(13 + i, pt)
    else:
        for tk in range(16):
            if len(pipe) >= 4:
                pv(tk - 4, pipe.pop(0))
            pipe.append(st_exp(tk))
            if tk % 4 == 3:
                vcb(tk // 4)
            elif tk in (5, 13) and fill:
                fill.pop(0)()
        for i, pt in enumerate(pipe):
            pv(12 + i, pt)
    assert not fill

    for h in range(2):
        # softmax sums (PSUM row 64) -> sums tile columns 512h..512h+512
        nc.vector.tensor_copy(
            sums[0:1, h * 512:(h + 1) * 512], ctx2[h][64:65, :])
        # ctx drain with bf16 cast (h1 shifts base 0 -> 64)
        nc.vector.tensor_copy(
            E.ctxT[h * 64:(h + 1) * 64, tq0:tq0 + 512], ctx2[h][0:64, :])
    return (tq0, sums)


def _norm(E, tq0, sums):
    """Normalization + V-bias for one 512-token group (DVE/gpsimd only)."""
    nc = E.nc
    bcast = E.bc_pool.tile([128, 512], FP32, tag="bcast", bufs=1)
    bcb = E.bc_pool.tile([128, 512], FP32, tag="bcb", bufs=1)
    nc.gpsimd.partition_broadcast(bcast[0:64, :], sums[0:1, 0:512])
    nc.gpsimd.partition_broadcast(bcb[0:64, :], sums[0:1, 512:1024])
    nc.vector.tensor_copy(bcast[64:128, :], bcb[0:64, :])
    recipb = E.bc_pool.tile([128, 512], FP32, tag="recipb", bufs=1)
    nc.vector.reciprocal_approx_fast(recipb[:], bcast[:])
    nc.vector.tensor_mul(E.ctxT[:, tq0:tq0 + 512], E.ctxT[:, tq0:tq0 + 512], recipb[:])
    nc.vector.tensor_scalar_add(
        E.ctxT[:, tq0:tq0 + 512], E.ctxT[:, tq0:tq0 + 512], E.bv_sb[:])


def _norm_tail(E, tq0, sums):
    """Tail-path normalization: the per-head reciprocal broadcast runs as
    one tiny PE matmul (mask-select from partitions 0/32) instead of two
    serial ~1.2us gpsimd PartitionBroadcasts; the st PSUM bufs are idle
    after the last group's exp, so one is borrowed for the broadcast."""
    nc = E.nc
    rcp = E.bc_pool.tile([1, 1024], FP32, tag="sums", bufs=4, name="rcpt")
    nc.vector.reciprocal_approx_fast(rcp[0:1, :], sums[0:1, :])
    bcps = E.st_ps.tile([128, 512], FP32, tag="st", name="bcps")
    nc.tensor.matmul(
        bcps[:], E.nmask[0:1, 0:128], rcp[0:1, 0:512], start=True, stop=False)
    nc.tensor.matmul(
        bcps[:], E.nmask[0:1, 128:256], rcp[0:1, 512:1024],
        start=False, stop=True)
    nc.vector.tensor_mul(E.ctxT[:, tq0:tq0 + 512], E.ctxT[:, tq0:tq0 + 512],
                         bcps[:])
    nc.vector.tensor_scalar_add(
        E.ctxT[:, tq0:tq0 + 512], E.ctxT[:, tq0:tq0 + 512], E.bv_sb[:])


def _outproj_half(E, tq0, half2, out):
    """Output projection for one 256-token half of a 512-token group.
    Both Wo halves of a 128-token block share one staging tile so the
    block goes out as a single dma_start (Sync descriptor issue is the
    scarce resource, ~610ns per dma_start)."""
    nc = E.nc
    for tc4 in (2 * half2, 2 * half2 + 1):
        t0 = tq0 + tc4 * 128
        ostg = E.ostg_pool.tile([128, 1024], FP16, tag="ostg", bufs=4)
        for half in range(2):
            ops = E.po_ps.tile([128, 512], FP32, tag="po", name="ops")
            nc.tensor.matmul(
                ops[:],
                E.ctxT[:, t0:t0 + 128],
                E.wo_sb[:, half * 512:(half + 1) * 512],
                start=True,
                stop=True,
            )
            nc.vector.tensor_copy(
                ostg[:, half * 512:(half + 1) * 512], ops[:])
        nc.sync.dma_start(out.ap()[t0:t0 + 128, :], ostg[:])


# ---------------- host-side helpers ----------------

def core_inputs(q, k, v, Wq, bq_, Wk, bk_, Wv, bv_, Wo, core):
    """Build the per-core input map (numpy, host-side shard/layout prep)."""
    import ml_dtypes

    bf16 = ml_dtypes.bfloat16
    dsl = slice(core * PH, (core + 1) * PH)

    def warr(W):
        # [1024, 128] slice -> [128 part, KC*128] (kc-major per partition)
        w = W[:, dsl].reshape(KC, 128, PH).transpose(1, 0, 2).reshape(128, KC * PH)
        return np.ascontiguousarray(w).astype(bf16)

    return {
        "wq": warr(Wq),
        "wk": warr(Wk),
        "wv": warr(Wv),
        "wo": np.ascontiguousarray(Wo[dsl, :]).astype(bf16),
        "bq": np.ascontiguousarray(bq_[dsl]).reshape(PH, 1).astype(np.float32),
        "bk": np.ascontiguousarray(bk_[dsl]).reshape(PH, 1).astype(np.float32),
        "bv": np.ascontiguousarray(bv_[dsl]).reshape(PH, 1).astype(np.float32),
    }


def shared_inputs(q, k, v):
    import ml_dtypes

    bf16 = ml_dtypes.bfloat16
    qT_np = np.ascontiguousarray(q.reshape(N, D).T).astype(bf16)
    kT_np = np.ascontiguousarray(k.reshape(N, D).T).astype(bf16)
    vT_np = np.ascontiguousarray(v.reshape(N, D).T).astype(bf16)
    return {"qT": qT_np, "kT": kT_np, "vT": vT_np}


# ---------------- public entry point ----------------

_NC_CACHE = []


def _get_nc():
    if not _NC_CACHE:
        _NC_CACHE.append(build())
    return _NC_CACHE[0]


def kernel(q, k, v, Wq, bq, Wk, bk, Wv, bv, Wo, bo):
    from concourse import bass_utils

    q = np.asarray(q, np.float32)
    k = np.asarray(k, np.float32)
    v = np.asarray(v, np.float32)
    Wq, bq = np.asarray(Wq, np.float32), np.asarray(bq, np.float32)
    Wk, bk = np.asarray(Wk, np.float32), np.asarray(bk, np.float32)
    Wv, bv = np.asarray(Wv, np.float32), np.asarray(bv, np.float32)
    Wo, bo = np.asarray(Wo, np.float32), np.asarray(bo, np.float32)

    nc = _get_nc()
    shared = shared_inputs(q, k, v)
    in_maps = []
    for core in range(8):
        m = dict(shared)
        m.update(core_inputs(q, k, v, Wq, bq, Wk, bk, Wv, bv, Wo, core))
        in_maps.append(m)

    res = bass_utils.run_bass_kernel_spmd(nc, in_maps, core_ids=list(range(8)))

    acc = np.zeros((N, D), np.float64)
    for r in res.results:
        acc += r["out"].astype(np.float64)
    outp = (acc + bo.astype(np.float64)).astype(np.float32)
    return outp.reshape(B, T, D)


# revision 32
# speedup vs baseline: 1.1992x; 1.0250x over previous
"""Self-contained TRN2 Bass kernel for 16-head MHA (B=2, T=2048, D=1024),
head-parallel across 8 NeuronCores (2 heads per core).

kernel(**inputs) takes the FULL fp32 inputs of reference.setup_inputs() and
returns the FULL [2, 2048, 1024] fp32 output.  Host-side prep: q/k/v are
transposed to [1024, 4096] bf16 (shared by all cores); each core gets its
128-column slice of Wq/Wk/Wv (and 128-row slice of Wo) in bf16.  Each core
computes its two heads end-to-end (QKV projections, softmax attention with
row-group-packed score matmuls, ones-augmented PV for free softmax sums,
output projection) and DMAs a rank-128 partial of the output back; the host
sums the 8 partials and adds the output bias.

v2: PV-before-S^T emission (hides PV weight loads under the score stream),
fine-grained t-major DMA for batch-0 k/q/v so attention starts as soon as
the first q chunk is projected, piecewise V projection threaded into the
first attention group of each batch, and explicit per-group filler lists
that leave only the last group's normalization + output projection in the
tail.
"""

import numpy as np

import concourse.bass as bass
import concourse.mybir as mybir
import concourse.tile as tile
from concourse import bacc

FP32 = mybir.dt.float32
FP16 = mybir.dt.float16
BF16 = mybir.dt.bfloat16

D = 1024          # model dim
N = 4096          # B*T tokens
B = 2
T = 2048
PH = 128          # per-core projection dims (2 heads x 64)
DH = 64           # head dim
KC = 8            # contraction chunks (1024 / 128)
NTC = N // 128    # 32 token chunks of 128
SCALE = 0.125     # 1/sqrt(64)

ACT_EXP = mybir.ActivationFunctionType.Exp


def build(nc=None):
    if nc is None:
        nc = bacc.Bacc(
            "TRN2",
            target_bir_lowering=False,
            debug=False,
            enable_asserts=False,
            num_devices=8,
        )

    # host pre-tiles q/k/v as [8 token-chunks, 128 part, KC*512] so each
    # 512-token mega tile is one fully-contiguous dma_start with 8KB
    # per-partition descriptors (the flat [D, N] layout only gave 1KB
    # descriptor rows, capping DMA at ~200 GB/s)
    qT = nc.dram_tensor("qT", [2 * 4, 128, KC * 512], BF16, kind="ExternalInput")
    kT = nc.dram_tensor("kT", [2 * 4, 128, KC * 512], BF16, kind="ExternalInput")
    vT = nc.dram_tensor("vT", [2 * 4, 128, KC * 512], BF16, kind="ExternalInput")
    # host pre-arranges W* into the [128, KC, 128] SBUF layout so the
    # weight DMA is one contiguous 2 KB-per-partition copy
    wq = nc.dram_tensor("wq", [128, KC * PH], BF16, kind="ExternalInput")
    wk = nc.dram_tensor("wk", [128, KC * PH], BF16, kind="ExternalInput")
    wv = nc.dram_tensor("wv", [128, KC * PH], BF16, kind="ExternalInput")
    wo = nc.dram_tensor("wo", [PH, D], BF16, kind="ExternalInput")
    bq = nc.dram_tensor("bq", [PH, 1], FP32, kind="ExternalInput")
    bk = nc.dram_tensor("bk", [PH, 1], FP32, kind="ExternalInput")
    bv = nc.dram_tensor("bv", [PH, 1], FP32, kind="ExternalInput")
    out = nc.dram_tensor("out", [N, D], FP16, kind="ExternalOutput")

    with tile.TileContext(nc) as tc:
        _emit(nc, tc, qT, kT, vT, wq, wk, wv, wo, bq, bk, bv, out)

    nc.compile()
    return nc


class _Ctx:
    pass


def _emit(nc, tc, qT, kT, vT, wq, wk, wv, wo, bq, bk, bv, out):
    from contextlib import ExitStack

    E = _Ctx()
    E.nc = nc

    ctxmgr = ExitStack()
    with ctxmgr:
        const_pool = ctxmgr.enter_context(tc.tile_pool(name="const", bufs=1))
        # batch-0 [128, KC, 512] t-major mega tiles: ONE dma_start each
        # (the Sync engine issues DMA descriptors at ~610ns per dma_start,
        # so the priming path must use as few dma_starts as possible)
        E.xm_pool = ctxmgr.enter_context(tc.tile_pool(name="xm", bufs=12))
        big_pool = ctxmgr.enter_context(tc.tile_pool(name="big", bufs=1))
        E.pt_pool = ctxmgr.enter_context(tc.tile_pool(name="pt", bufs=6))
        E.bc_pool = ctxmgr.enter_context(tc.tile_pool(name="bc", bufs=1))
        E.vt_pool = ctxmgr.enter_context(tc.tile_pool(name="vt", bufs=2))
        E.ostg_pool = ctxmgr.enter_context(tc.tile_pool(name="ostg", bufs=4))
        # PSUM: proj/outproj/transpose 2 banks + st 4 + ctx 2 = 8
        E.po_ps = ctxmgr.enter_context(
            tc.tile_pool(name="po_ps", bufs=2, space="PSUM"))
        E.st_ps = ctxmgr.enter_context(
            tc.tile_pool(name="st_ps", bufs=2, space="PSUM"))
        E.ctx_ps = ctxmgr.enter_context(
            tc.tile_pool(name="ctx_ps", bufs=2, space="PSUM"))

        # --- weights / consts to SBUF ---
        wq_sb = const_pool.tile([128, KC, PH], BF16, tag="wq")
        wk_sb = const_pool.tile([128, KC, PH], BF16, tag="wk")
        wv_sb = const_pool.tile([128, KC, PH], BF16, tag="wv")
        E.wo_sb = const_pool.tile([128, D], BF16, tag="wo")
        bq_sb = const_pool.tile([128, 1], FP32, tag="bq")
        bk_sb = const_pool.tile([128, 1], FP32, tag="bk")
        E.bv_sb = const_pool.tile([128, 1], FP32, tag="bv")
        nc.sync.dma_start(wk_sb[:], wk.ap().rearrange("p (c m) -> p c m", c=KC))
        nc.sync.dma_start(bk_sb[:], bk.ap())

        # persistent activations
        E.qT_sb = big_pool.tile([128, N], BF16, tag="qTsb")
        E.kT_sb = big_pool.tile([128, N], BF16, tag="kTsb")
        # v_aug pair layout: [tok part, 32 tok chunks, 130]; per head h the
        # PV stationary operand is vp[:, chunk, 65h : 65h+65] = [v_h | ones]
        E.vp = big_pool.tile([128, NTC, 130], BF16, tag="vp")
        E.ctxT = big_pool.tile([128, N], BF16, tag="ctxT")

        nc.gpsimd.memset(E.vp[:, :, 64], 1.0)
        nc.gpsimd.memset(E.vp[:, :, 129], 1.0)

        E.identity = const_pool.tile([128, 128], BF16, tag="ident")
        from concourse.masks import make_identity
        make_identity(nc, E.identity[:])

        # masks for the tail-normalization PE broadcast (all partition-0):
        # cols 0:128 select out partitions 0:64 (head 0), cols 128:256
        # select out partitions 64:128 (head 1)
        E.nmask = const_pool.tile([128, 256], FP16, tag="nmask")
        nc.gpsimd.memset(E.nmask[:], 0.0)
        nc.gpsimd.memset(E.nmask[0:1, 0:64], 1.0)
        nc.gpsimd.memset(E.nmask[0:1, 192:256], 1.0)

        def dma_fine(nm, xdram, b, t):
            """One t-major [128, KC, 512] mega tile: a single dma_start
            copying one fully-contiguous dram block, so a proj chunk can
            start as soon as its own 1MB lands.  Returns per-kc APs."""
            xm = E.xm_pool.tile(
                [128, KC, 512], BF16, tag="xm", bufs=12,
                name=f"xm_{nm}{b}{t}")
            nc.sync.dma_start(
                xm[:],
                xdram.ap()[b * 4 + t].rearrange("p (c m) -> p c m", c=KC))
            return [xm[:, kc, :] for kc in range(KC)]

        def group(b, tqc, vcb=None, fillers=(), tail=False):
            return _attention_group(E, b, tqc, vcb, list(fillers), tail)

        def pchunk_f(xts, wsb, bias_sb, dstT, b, t):
            # fine tile set: columns 0:512 of the set are exactly chunk t
            return lambda: _proj_chunk(E, xts, wsb, bias_sb, dstT, b * T, t,
                                       tile_t0=t)

        def unorm(pend, half2, tail=False):
            # half2==0 also runs the (cheap, DVE-side) normalization
            tq0, sums = pend

            def run():
                if half2 == 0:
                    if tail:
                        _norm_tail(E, tq0, sums)
                    else:
                        _norm(E, tq0, sums)
                _outproj_half(E, tq0, half2, out, tail)
            return run

        def make_vcb(getx, b):
            """Piecewise V projection: chunk j projects tokens [512j,512j+512)
            of batch b and transposes them into vp, so PV(tk) only needs
            chunk tk//4 done.  getx(j, kc) returns the [128,512] input AP."""
            def vcb(j):
                vt = E.vt_pool.tile([128, 512], BF16, tag="vt", bufs=2,
                                    name=f"vt{b}{j}")
                ps = E.po_ps.tile([128, 512], FP32, tag="po", name="vps")
                for kc in range(KC):
                    nc.tensor.matmul(
                        ps[:], wv_sb[:, kc, :], getx(j, kc),
                        start=(kc == 0), stop=(kc == KC - 1))
                nc.vector.tensor_copy(vt[:], ps[:])
                for tloc in range(4):
                    tcid = b * 16 + j * 4 + tloc
                    tr = E.po_ps.tile([128, 128], BF16, tag="po",
                                      name=f"tr{tcid}")
                    nc.tensor.transpose(
                        tr[:], vt[:, tloc * 128:(tloc + 1) * 128], E.identity[:])
                    nc.vector.tensor_copy(E.vp[:, tcid, 0:64], tr[:, 0:64])
                    nc.vector.tensor_copy(E.vp[:, tcid, 65:129], tr[:, 64:128])
            return vcb

        # ---- priming: batch-0 DMAs fine-grained t-major, ordered so each
        # consumer's megabyte lands just in time: k (proj now), q chunk 0
        # (first group), then v chunks (vcb slots inside group 0), then the
        # remaining q chunks (fillers in group 1).
        xk0 = [dma_fine("k", kT, 0, t) for t in range(4)]
        nc.sync.dma_start(wq_sb[:], wq.ap().rearrange("p (c m) -> p c m", c=KC))
        nc.sync.dma_start(bq_sb[:], bq.ap())
        xq0_0 = dma_fine("q", qT, 0, 0)
        nc.sync.dma_start(wv_sb[:], wv.ap().rearrange("p (c m) -> p c m", c=KC))
        nc.sync.dma_start(E.wo_sb[:], wo.ap())
        nc.sync.dma_start(E.bv_sb[:], bv.ap())
        xv0 = [dma_fine("v", vT, 0, t) for t in range(4)]
        xq0r = {t: dma_fine("q", qT, 0, t) for t in (1, 2, 3)}

        for t in range(4):
            _proj_chunk(E, xk0[t], wk_sb, bk_sb, E.kT_sb, 0, t, tile_t0=t,
                        drain_act=True)
        _proj_chunk(E, xq0_0, wq_sb, bq_sb, E.qT_sb, 0, 0, tile_t0=0,
                    drain_act=True)

        g00 = group(0, 0, vcb=make_vcb(lambda j, kc: xv0[j][kc], 0))
        xk1 = [dma_fine("k", kT, 1, t) for t in range(4)]
        _proj_chunk(E, xq0r[1], wq_sb, bq_sb, E.qT_sb, 0, 1, tile_t0=1)
        g01 = group(0, 1, fillers=[
            pchunk_f(xq0r[2], wq_sb, bq_sb, E.qT_sb, 0, 2),
            pchunk_f(xq0r[3], wq_sb, bq_sb, E.qT_sb, 0, 3),
            pchunk_f(xk1[0], wk_sb, bk_sb, E.kT_sb, 1, 0),
            pchunk_f(xk1[1], wk_sb, bk_sb, E.kT_sb, 1, 1),
        ])
        xq1 = [dma_fine("q", qT, 1, t) for t in range(4)]
        g02 = group(0, 2, fillers=[
            pchunk_f(xk1[2], wk_sb, bk_sb, E.kT_sb, 1, 2),
            pchunk_f(xk1[3], wk_sb, bk_sb, E.kT_sb, 1, 3),
            unorm(g00, 0),
            unorm(g00, 1),
        ])
        g03 = group(0, 3, fillers=[
            pchunk_f(xq1[0], wq_sb, bq_sb, E.qT_sb, 1, 0),
            unorm(g01, 0),
            unorm(g01, 1),
        ])
        xv1 = [dma_fine("v", vT, 1, t) for t in range(4)]
        g10 = group(1, 0, vcb=make_vcb(lambda j, kc: xv1[j][kc], 1),
            fillers=[
                pchunk_f(xq1[1], wq_sb, bq_sb, E.qT_sb, 1, 1),
                unorm(g02, 0),
            ])
        g11 = group(1, 1, fillers=[
            pchunk_f(xq1[2], wq_sb, bq_sb, E.qT_sb, 1, 2),
            pchunk_f(xq1[3], wq_sb, bq_sb, E.qT_sb, 1, 3),
            unorm(g02, 1),
            unorm(g03, 0),
        ])
        g12 = group(1, 2, fillers=[
            unorm(g03, 1),
            unorm(g10, 0),
            unorm(g10, 1),
            unorm(g11, 0),
        ])
        g13 = group(1, 3, fillers=[
            unorm(g11, 1),
            unorm(g12, 0),
            unorm(g12, 1),
        ], tail=True)
        unorm(g13, 0, tail=True)()
        unorm(g13, 1, tail=True)()


def _proj_chunk(E, xts, wsb, bias_sb, dstT, btok, t, tile_t0=0,
                drain_act=False):
    """One 512-token projection chunk: accumulate 8 kc matmuls, drain.
    xts tiles start at token column tile_t0*512 of this batch."""
    nc = E.nc
    ps = E.po_ps.tile([128, 512], FP32, tag="po", name="ps")
    c0 = (t - tile_t0) * 512
    for kc in range(KC):
        nc.tensor.matmul(
            ps[:],
            wsb[:, kc, :],
            xts[kc][:, c0:c0 + 512],
            start=(kc == 0),
            stop=(kc == KC - 1),
        )
    dst = dstT[:, btok + t * 512: btok + (t + 1) * 512]
    if drain_act:
        # ScalarE drain (idle during priming); Identity has a free
        # per-partition bias add
        if bias_sb is not None:
            nc.scalar.activation(
                dst, ps[:], mybir.ActivationFunctionType.Identity, bias=bias_sb[:])
        else:
            nc.scalar.activation(dst, ps[:], mybir.ActivationFunctionType.Identity)
    elif bias_sb is not None:
        nc.vector.tensor_scalar_add(dst, ps[:], bias_sb[:])
    else:
        nc.vector.tensor_copy(dst, ps[:])


def _attention_group(E, b, tqc, vcb, fillers, tail=False):
    """S^T/exp/PV + sums & ctx drains for one 512-token group (both heads).

    The two heads' S^T matmuls are row-group packed: head h's K=64
    contraction occupies array rows 64h..64h+63, so the pair runs
    concurrently on the PE.  st is one flat [128, 1024] PSUM tile (2 banks,
    head h in columns 512h..512h+512) so the exp is a single contiguous
    2-D ACTIVATE over 1024 elements per partition.

    PV(tk-3) is emitted BEFORE S^T(tk): each matmul's weight load then
    hides under the previous matmul's moving stream (the PV loads are
    128 rows and do not fit behind their own pair's stream otherwise).
    Filler units (deferred projections, norm+output-projection halves)
    are paced into the exp-bound slack at fixed tk slots.

    With vcb set (first group of each batch), vcb(j) is emitted at
    tk = 4j+3 to project+transpose V tokens [512j, 512j+512), and PV
    trails by 4 so PV(tk) always finds its vp chunk ready.
    """
    nc = E.nc
    btok = b * T
    tq0 = btok + tqc * 512

    # both heads' softmax sums side by side in the free dim (partition
    # offsets must be 32-aligned, so two partitions is not an option)
    sums = E.bc_pool.tile([1, 1024], FP32, tag="sums", bufs=4,
                          name=f"sums{b}{tqc}")
    ctx2 = [
        E.ctx_ps.tile([65, 512], FP32, tag="ctx", name=f"ctx{h}")
        for h in range(2)
    ]

    def st_exp(tk):
        st = E.st_ps.tile([128, 1024], FP32, tag="st", name="st")
        for h in range(2):
            nc.tensor.matmul(
                st[:, h * 512:(h + 1) * 512],
                E.kT_sb[h * 64:(h + 1) * 64,
                        btok + tk * 128: btok + (tk + 1) * 128],
                E.qT_sb[h * 64:(h + 1) * 64, tq0:tq0 + 512],
                start=True,
                stop=True,
            )
        pt = E.pt_pool.tile([128, 1024], BF16, tag="pt", name="pt")
        nc.scalar.activation(pt[:], st[:], ACT_EXP, scale=SCALE)
        return pt

    def pv(tk, pt):
        for h in range(2):
            nc.tensor.matmul(
                ctx2[h][:],
                E.vp[:, b * 16 + tk, h * 65:(h + 1) * 65],
                pt[:, h * 512:(h + 1) * 512],
                start=(tk == 0),
                stop=(tk == 15),
            )

    fill = list(fillers)
    pipe = []
    if vcb is None:
        for tk in range(16):
            if len(pipe) >= 3:
                pv(tk - 3, pipe.pop(0))
            pipe.append(st_exp(tk))
            if tk in (3, 7, 11, 15) and fill:
                fill.pop(0)()
        for i, pt in enumerate(pipe):
            pv(13 + i, pt)
    else:
        for tk in range(16):
            if len(pipe) >= 4:
                pv(tk - 4, pipe.pop(0))
            pipe.append(st_exp(tk))
            if tk % 4 == 3:
                vcb(tk // 4)
            elif tk in (5, 13) and fill:
                fill.pop(0)()
        for i, pt in enumerate(pipe):
            pv(12 + i, pt)
    assert not fill

    for h in range(2):
        # softmax sums (PSUM row 64) -> sums tile columns 512h..512h+512
        nc.vector.tensor_copy(
            sums[0:1, h * 512:(h + 1) * 512], ctx2[h][64:65, :])
        # ctx drain with bf16 cast (h1 shifts base 0 -> 64); in the tail
        # the drain runs on the (idle) scalar engine so the DVE can start
        # on the reciprocal immediately
        dst = E.ctxT[h * 64:(h + 1) * 64, tq0:tq0 + 512]
        if tail:
            nc.scalar.activation(
                dst, ctx2[h][0:64, :], mybir.ActivationFunctionType.Identity)
        else:
            nc.vector.tensor_copy(dst, ctx2[h][0:64, :])
    return (tq0, sums)


def _norm(E, tq0, sums):
    """Normalization + V-bias for one 512-token group (DVE/gpsimd only)."""
    nc = E.nc
    bcast = E.bc_pool.tile([128, 512], FP32, tag="bcast", bufs=1)
    bcb = E.bc_pool.tile([128, 512], FP32, tag="bcb", bufs=1)
    nc.gpsimd.partition_broadcast(bcast[0:64, :], sums[0:1, 0:512])
    nc.gpsimd.partition_broadcast(bcb[0:64, :], sums[0:1, 512:1024])
    nc.vector.tensor_copy(bcast[64:128, :], bcb[0:64, :])
    recipb = E.bc_pool.tile([128, 512], FP32, tag="recipb", bufs=1)
    nc.vector.reciprocal_approx_fast(recipb[:], bcast[:])
    nc.vector.tensor_mul(E.ctxT[:, tq0:tq0 + 512], E.ctxT[:, tq0:tq0 + 512], recipb[:])
    nc.vector.tensor_scalar_add(
        E.ctxT[:, tq0:tq0 + 512], E.ctxT[:, tq0:tq0 + 512], E.bv_sb[:])


def _norm_tail(E, tq0, sums):
    """Tail-path normalization: the per-head reciprocal broadcast runs as
    one tiny PE matmul (mask-select from partitions 0/32) instead of two
    serial ~1.2us gpsimd PartitionBroadcasts; the st PSUM bufs are idle
    after the last group's exp, so one is borrowed for the broadcast."""
    nc = E.nc
    rcp32 = E.bc_pool.tile([1, 1024], FP32, tag="sums", bufs=4, name="rcp32")
    nc.vector.reciprocal_approx_fast(rcp32[0:1, :], sums[0:1, :])
    rcp = E.bc_pool.tile([1, 1024], FP16, tag="rcpt", bufs=1, name="rcpt")
    nc.vector.tensor_copy(rcp[0:1, :], rcp32[0:1, :])
    bcps = E.st_ps.tile([128, 512], FP32, tag="st", name="bcps")
    nc.tensor.matmul(
        bcps[:], E.nmask[0:1, 0:128], rcp[0:1, 0:512], start=True, stop=False)
    nc.tensor.matmul(
        bcps[:], E.nmask[0:1, 128:256], rcp[0:1, 512:1024],
        start=False, stop=True)
    nc.vector.tensor_mul(E.ctxT[:, tq0:tq0 + 512], E.ctxT[:, tq0:tq0 + 512],
                         bcps[:])
    nc.vector.tensor_scalar_add(
        E.ctxT[:, tq0:tq0 + 512], E.ctxT[:, tq0:tq0 + 512], E.bv_sb[:])


def _outproj_half(E, tq0, half2, out, tail=False):
    """Output projection for one 256-token half of a 512-token group.
    Both Wo halves of a 128-token block share one staging tile so the
    block goes out as a single dma_start (Sync descriptor issue is the
    scarce resource, ~610ns per dma_start).  In the tail, PSUM tiles
    alternate po_ps/ctx_ps (the ctx accumulators are free) and drains
    run on the scalar engine, so the four matmuls stream back-to-back
    instead of ping-ponging against DVE casts."""
    nc = E.nc
    for i, tc4 in enumerate((2 * half2, 2 * half2 + 1)):
        t0 = tq0 + tc4 * 128
        ostg = E.ostg_pool.tile([128, 1024], FP16, tag="ostg", bufs=4)
        for half in range(2):
            if tail and (2 * i + half) % 2 == 1:
                ops = E.ctx_ps.tile([128, 512], FP32, tag="ctx", name="ops")
            else:
                ops = E.po_ps.tile([128, 512], FP32, tag="po", name="ops")
            nc.tensor.matmul(
                ops[:],
                E.ctxT[:, t0:t0 + 128],
                E.wo_sb[:, half * 512:(half + 1) * 512],
                start=True,
                stop=True,
            )
            dst = ostg[:, half * 512:(half + 1) * 512]
            if tail:
                nc.scalar.activation(
                    dst, ops[:], mybir.ActivationFunctionType.Identity)
            else:
                nc.vector.tensor_copy(dst, ops[:])
        nc.sync.dma_start(out.ap()[t0:t0 + 128, :], ostg[:])


# ---------------- host-side helpers ----------------

def core_inputs(q, k, v, Wq, bq_, Wk, bk_, Wv, bv_, Wo, core):
    """Build the per-core input map (numpy, host-side shard/layout prep)."""
    import ml_dtypes

    bf16 = ml_dtypes.bfloat16
    dsl = slice(core * PH, (core + 1) * PH)

    def warr(W):
        # [1024, 128] slice -> [128 part, KC*128] (kc-major per partition)
        w = W[:, dsl].reshape(KC, 128, PH).transpose(1, 0, 2).reshape(128, KC * PH)
        return np.ascontiguousarray(w).astype(bf16)

    return {
        "wq": warr(Wq),
        "wk": warr(Wk),
        "wv": warr(Wv),
        "wo": np.ascontiguousarray(Wo[dsl, :]).astype(bf16),
        "bq": np.ascontiguousarray(bq_[dsl]).reshape(PH, 1).astype(np.float32),
        "bk": np.ascontiguousarray(bk_[dsl]).reshape(PH, 1).astype(np.float32),
        "bv": np.ascontiguousarray(bv_[dsl]).reshape(PH, 1).astype(np.float32),
    }


def shared_inputs(q, k, v):
    import ml_dtypes

    bf16 = ml_dtypes.bfloat16

    def tiled(x):
        # [N, D] -> [8 token-chunks, 128 part, KC, 512] so each 512-token
        # mega tile is one contiguous dram block (8KB per partition)
        xT = x.reshape(N, D).T                       # [D, N]
        arr = xT.reshape(KC, 128, 8, 512).transpose(2, 1, 0, 3)
        return np.ascontiguousarray(arr.reshape(8, 128, KC * 512)).astype(bf16)

    return {"qT": tiled(q), "kT": tiled(k), "vT": tiled(v)}


# ---------------- public entry point ----------------

_NC_CACHE = []


def _get_nc():
    if not _NC_CACHE:
        _NC_CACHE.append(build())
    return _NC_CACHE[0]


def kernel(q, k, v, Wq, bq, Wk, bk, Wv, bv, Wo, bo):
    from concourse import bass_utils

    q = np.asarray(q, np.float32)
    k = np.asarray(k, np.float32)
    v = np.asarray(v, np.float32)
    Wq, bq = np.asarray(Wq, np.float32), np.asarray(bq, np.float32)
    Wk, bk = np.asarray(Wk, np.float32), np.asarray(bk, np.float32)
    Wv, bv = np.asarray(Wv, np.float32), np.asarray(bv, np.float32)
    Wo, bo = np.asarray(Wo, np.float32), np.asarray(bo, np.float32)

    nc = _get_nc()
    shared = shared_inputs(q, k, v)
    in_maps = []
    for core in range(8):
        m = dict(shared)
        m.update(core_inputs(q, k, v, Wq, bq, Wk, bk, Wv, bv, Wo, core))
        in_maps.append(m)

    res = bass_utils.run_bass_kernel_spmd(nc, in_maps, core_ids=list(range(8)))

    acc = np.zeros((N, D), np.float64)
    for r in res.results:
        acc += r["out"].astype(np.float64)
    outp = (acc + bo.astype(np.float64)).astype(np.float32)
    return outp.reshape(B, T, D)
